# revision 1
# baseline (speedup 1.0000x reference)
"""Causal-self-attention (non-causal SDPA + RoPE) Bass kernel for 8 Trainium2 cores.

Sharding: head-parallel. 16 heads / 8 cores = 2 heads per core, all 4 batches.
Each core computes QKV projections for its 2 heads (tensor-parallel split of
Wqkv rows), RoPE, full attention for its 8 (batch, head) units, and a partial
output projection against its 128-column slice of Wout. The 8 partial outputs
are summed on the host (the all-reduce of the tensor-parallel out-proj).

Layouts on device (per core):
  xt      [1024, 8192]  X^T, f32r; column m = b*2048 + t (b-major)
  qt/kt   [128, 8192]   Q^T/K^T after RoPE; rows = 2 heads x 64 dims
  v       [128, 16*130] per batch: 16 s-tiles of [128s, 65+65] = [V_A|1 | V_B|1], bf16
  scores  S^T[s, t] via row-packed K=64 matmuls (2 heads concurrent on the PE)
  exp     ScalarE, scale=0.125 fused, no max-subtraction (scores ~ N(0,1))
  PV      attn^T[d, t] = [V|1].T @ E^T ; M=65 -> row 64 = softmax denominator
  outproj bf16: Wout slice^T as lhsT, scaled attn^T as rhs -> out^T [f, m]

Emission is software-pipelined per (batch, t-window): attention(b, tc) is
followed by one quarter of proj(b+1) and one quarter of outproj(b-1), so the
PE fills ScalarE-bound attention windows with projection work and ScalarE
never starves at batch boundaries.
"""

import numpy as np

EMBED = 1024
NUM_HEADS = 16
HEAD_DIM = 64
T = 2048
B = 4
NCORES = 8
M = T * B  # 8192
ROPE_BASE = 10000.0


def _build_program():
    import concourse.bass as bass  # noqa: F401
    import concourse.mybir as mybir
    import concourse.tile as tile
    from concourse import bacc

    dt = mybir.dt
    F32, F32R, BF16 = dt.float32, dt.float32r, dt.bfloat16
    AF = mybir.ActivationFunctionType

    nc = bacc.Bacc("TRN2", target_bir_lowering=False, debug=False,
                   num_devices=NCORES)

    xt = nc.dram_tensor("xt", [EMBED, M], F32R, kind="ExternalInput")
    wq = nc.dram_tensor("wq", [EMBED, 128], F32R, kind="ExternalInput")
    wk = nc.dram_tensor("wk", [EMBED, 128], F32R, kind="ExternalInput")
    wv = nc.dram_tensor("wv", [EMBED, 128], F32R, kind="ExternalInput")
    wo = nc.dram_tensor("wo", [128, EMBED], BF16, kind="ExternalInput")
    cosd = nc.dram_tensor("cosd", [128, T], F32, kind="ExternalInput")
    sind = nc.dram_tensor("sind", [128, T], F32, kind="ExternalInput")
    identd = nc.dram_tensor("identd", [128, 128], F32R, kind="ExternalInput")
    onesd = nc.dram_tensor("onesd", [1, 128], F32R, kind="ExternalInput")
    out = nc.dram_tensor("out", [EMBED, M], F32, kind="ExternalOutput")

    ST = 16            # s-tiles per batch (2048/128)
    VST = 130          # per-s-tile V columns: [V_A | 1 | V_B | 1]

    with tile.TileContext(nc) as tc:
        with (
            tc.tile_pool(name="const", bufs=1) as cpool,
            tc.tile_pool(name="xt", bufs=16) as xpool,
            tc.tile_pool(name="big", bufs=1) as big,
            tc.tile_pool(name="vt", bufs=2) as vtpool,
            tc.tile_pool(name="rt", bufs=2) as rtpool,
            tc.tile_pool(name="et", bufs=3) as epool,
            tc.tile_pool(name="sc", bufs=2) as scpool,
            tc.tile_pool(name="ob", bufs=3) as opool,
            tc.tile_pool(name="pp", bufs=2, space="PSUM") as pp,
            tc.tile_pool(name="ps", bufs=2, space="PSUM") as ps,
            tc.tile_pool(name="pa", bufs=2, space="PSUM") as pa,
        ):
            # ---- constants ----
            wq_sb = cpool.tile([128, 1024], F32R, tag="wq")
            wk_sb = cpool.tile([128, 1024], F32R, tag="wk")
            wv_sb = cpool.tile([128, 1024], F32R, tag="wv")
            for e in range(8):
                nc.sync.dma_start(wq_sb[:, e * 128:(e + 1) * 128],
                                  wq[e * 128:(e + 1) * 128, :])
                nc.sync.dma_start(wk_sb[:, e * 128:(e + 1) * 128],
                                  wk[e * 128:(e + 1) * 128, :])
                nc.sync.dma_start(wv_sb[:, e * 128:(e + 1) * 128],
                                  wv[e * 128:(e + 1) * 128, :])

            cos_sb = cpool.tile([128, T], F32, tag="cos")
            sin_sb = cpool.tile([128, T], F32, tag="sin")
            ident = cpool.tile([128, 128], F32R, tag="ident")
            ones1 = cpool.tile([1, 128], F32R, tag="ones")
            wo_sb = cpool.tile([128, 1024], BF16, tag="wo")

            def load_tables():
                nc.sync.dma_start(cos_sb[:], cosd[:])
                nc.sync.dma_start(sin_sb[:], sind[:])
                nc.sync.dma_start(ident[:], identd[:])
                nc.sync.dma_start(ones1[:], onesd[:])
                nc.sync.dma_start(wo_sb[:], wo[:])

            warm = cpool.tile([1, 128], F32, tag="warm")
            nc.scalar.activation(warm[:], ones1[:].bitcast(F32),
                                 AF.Exp, scale=0.0)
            qt_sb = big.tile([128, M], F32R, tag="qt")
            kt_sb = big.tile([128, M], F32R, tag="kt")
            v_sb = [big.tile([128, ST * VST], BF16, tag=f"v{b}", name=f"v_sb{b}")
                    for b in range(B)]
            attnS = {(b, g): rtpool.tile([128, 512], BF16, tag="attnS",
                                          name=f"attnS{b}_{g}")
                     for b in range(B) for g in range(4)}

            def load_x_half(h, mc):
                """Load 8 e-chunk tiles of X^T for 512 m-cols at h*1024+mc*512."""
                c0 = h * 1024 + mc * 512
                xts = [xpool.tile([128, 512], F32R, tag="xt",
                                  name=f"xt{h}_{mc}_{e}")
                       for e in range(8)]
                for e in range(8):
                    nc.sync.dma_start(xts[e][:],
                                      xt[e * 128:(e + 1) * 128, c0:c0 + 512])
                return xts

            def rope(p, dst, col0):
                """dst = cos*p + sin_eff*shift32(p), all [128, 512]."""
                pr = rtpool.tile([128, 512], F32, tag="proj_sb")
                nc.any.tensor_copy(pr[:], p[:])
                prs = rtpool.tile([128, 512], F32, tag="ropeshuf")
                for (ob, ib) in ((0, 32), (32, 0), (64, 96), (96, 64)):
                    nc.sync.dma_start(prs[ob:ob + 32, :], pr[ib:ib + 32, :])
                t2 = rtpool.tile([128, 512], F32, tag="ropetmp")
                tc0 = col0 % T
                nc.vector.tensor_mul(t2[:], prs[:], sin_sb[:, tc0:tc0 + 512])
                nc.vector.tensor_mul(dst, pr[:], cos_sb[:, tc0:tc0 + 512])
                nc.vector.tensor_add(dst, dst, t2[:])

            def proj_chunk(xts, h, mc):
                """Q/K/V projections + rope + V transpose for 512 m-columns."""
                b = h // 2
                col0 = h * 1024 + mc * 512
                for w_sb, dst in ((wq_sb, qt_sb), (wk_sb, kt_sb)):
                    p = pp.tile([128, 512], F32, tag="pp")
                    for e in range(8):
                        nc.tensor.matmul(
                            p[:], w_sb[:, e * 128:(e + 1) * 128], xts[e][:],
                            start=(e == 0), stop=(e == 7))
                    rope(p, dst[:, col0:col0 + 512], col0)
                p = pp.tile([128, 512], F32, tag="pp")
                for e in range(8):
                    nc.tensor.matmul(
                        p[:], wv_sb[:, e * 128:(e + 1) * 128], xts[e][:],
                        start=(e == 0), stop=(e == 7))
                vt = vtpool.tile([128, 512], F32R, tag="vt")
                nc.any.tensor_copy(vt[:], p[:])
                for k in range(4):
                    stt = (col0 % T) // 128 + k  # s-tile index 0..15
                    pt = pp.tile([128, 128], F32R, tag="pp")
                    nc.tensor.transpose(pt[:], vt[:, k * 128:(k + 1) * 128],
                                        ident[:])
                    # single strided copy: cols 0-63 -> +0, 64-127 -> +65
                    dstv = v_sb[b][:, stt * VST:stt * VST + 130]
                    nc.vector.tensor_copy(
                        dstv.rearrange("p (h c) -> p h c", c=65)[:, :, 0:64],
                        pt.rearrange("p (h c) -> p h c", c=64))

            def set_v_ones(b):
                nc.vector.memset(
                    v_sb[b].rearrange("p (s c) -> p s c", c=VST)[:, :, 64:65], 1.0)
                nc.vector.memset(
                    v_sb[b].rearrange("p (s c) -> p s c", c=VST)[:, :, 129:130], 1.0)

            def attention_tc(b, tcg):
                c0 = b * T + tcg * 512
                att_A = pa.tile([128, 512], F32, tag="pa")
                att_B = pa.tile([128, 512], F32, tag="pa")
                for st in range(ST):
                    s0 = b * T + st * 128
                    sab = ps.tile([128, 1024], F32, tag="sab")
                    nc.tensor.matmul(sab[:, 0:512],
                                     kt_sb[0:64, s0:s0 + 128],
                                     qt_sb[0:64, c0:c0 + 512],
                                     start=True, stop=True)
                    nc.tensor.matmul(sab[:, 512:1024],
                                     kt_sb[64:128, s0:s0 + 128],
                                     qt_sb[64:128, c0:c0 + 512],
                                     start=True, stop=True)
                    e_t = epool.tile([128, 1024], BF16, tag="et")
                    nc.scalar.activation(e_t[:], sab[:], AF.Exp, scale=0.125)
                    nc.tensor.matmul(att_A[0:65, :],
                                     v_sb[b][:, st * VST:st * VST + 65],
                                     e_t[:, 0:512],
                                     start=(st == 0), stop=(st == ST - 1))
                    nc.tensor.matmul(att_B[0:65, :],
                                     v_sb[b][:, st * VST + 65:st * VST + 130],
                                     e_t[:, 512:1024],
                                     start=(st == 0), stop=(st == ST - 1))
                # denominators -> reciprocal -> broadcast -> scale
                rec = scpool.tile([1, 1024], F32R, tag="rec")
                with nc.allow_low_precision(reason="softmax denom recip"):
                    nc.vector.reciprocal(rec[:, 0:512], att_A[64:65, :])
                    nc.vector.reciprocal(rec[:, 512:1024], att_B[64:65, :])
                for att, half in ((att_A, 0), (att_B, 1)):
                    bcp = pp.tile([128, 512], F32, tag="pp")
                    nc.tensor.matmul(bcp[:], ones1[:],
                                     rec[:, half * 512:(half + 1) * 512],
                                     start=True, stop=True)
                    bcs = scpool.tile([64, 512], F32, tag="bcs")
                    nc.any.tensor_copy(bcs[:], bcp[0:64, :])
                    nc.vector.tensor_mul(
                        attnS[(b, tcg)][half * 64:(half + 1) * 64, :],
                        att[0:64, :], bcs[:])

            def outproj_q(b, tcg, evict_engine=None):
                for ft in range(8):
                    po = pp.tile([128, 512], F32, tag="pp")
                    nc.tensor.matmul(po[:],
                                     wo_sb[:, ft * 128:(ft + 1) * 128],
                                     attnS[(b, tcg)][:],
                                     start=True, stop=True)
                    o_sb = opool.tile([128, 512], F32, tag="ob")
                    if evict_engine is None:
                        nc.vector.tensor_copy(o_sb[:], po[:])
                    else:
                        evict_engine.activation(
                            o_sb[:], po[:],
                            mybir.ActivationFunctionType.Copy)
                    nc.sync.dma_start(
                        out[ft * 128:(ft + 1) * 128,
                            b * T + tcg * 512:b * T + (tcg + 1) * 512],
                        o_sb[:])

            # ---- software-pipelined emission ----
            for b in range(B):
                set_v_ones(b)
            first = load_x_half(0, 0)
            load_tables()
            proj_chunk(first, 0, 0)
            for g in range(1, 4):
                xts = load_x_half(g // 2, g % 2)
                proj_chunk(xts, g // 2, g % 2)
            prev = None
            for b in range(B):
                for tcg in range(4):
                    attention_tc(b, tcg)
                    if b + 1 < B:
                        h, mc = 2 * (b + 1) + tcg // 2, tcg % 2
                        xts = load_x_half(h, mc)
                        proj_chunk(xts, h, mc)
                    if prev is not None:
                        outproj_q(*prev)
                    prev = (b, tcg)
            outproj_q(*prev, evict_engine=nc.scalar)

    nc.compile()
    return nc


def _host_prep(query, Wqkv, Wout):
    import ml_dtypes

    q32 = np.asarray(query, dtype=np.float32)
    # [T, B, E] -> [E, B, T] -> [E, B*T]  (column = b*T + t)
    xt = np.ascontiguousarray(q32.transpose(2, 1, 0).reshape(EMBED, M))

    # rope tables, fp16-rounded like the reference
    theta = np.power(ROPE_BASE,
                     -np.arange(0, HEAD_DIM, 2, dtype=np.float32) / HEAD_DIM)
    m_th = np.arange(T, dtype=np.float32)[:, None] * theta[None, :]
    m_th = np.concatenate([m_th, m_th], axis=-1)          # [T, 64]
    cos = np.cos(m_th).astype(np.float16).astype(np.float32)
    sin = np.sin(m_th).astype(np.float16).astype(np.float32)
    cosT = cos.T                                          # [64, T]
    sin_eff = sin.T.copy()
    sin_eff[0:32] = -sin_eff[0:32]
    cos128 = np.ascontiguousarray(np.concatenate([cosT, cosT], axis=0))
    sin128 = np.ascontiguousarray(np.concatenate([sin_eff, sin_eff], axis=0))

    W = np.asarray(Wqkv, dtype=np.float32)
    Wo = np.asarray(Wout, dtype=np.float32)
    in_maps = []
    for c in range(NCORES):
        sl = slice(c * 128, (c + 1) * 128)
        in_maps.append({
            "xt": xt,
            "wq": np.ascontiguousarray(W[sl, :].T),
            "wk": np.ascontiguousarray(W[EMBED:][sl, :].T),
            "wv": np.ascontiguousarray(W[2 * EMBED:][sl, :].T),
            "wo": np.ascontiguousarray(Wo[:, sl].T).astype(ml_dtypes.bfloat16),
            "cosd": cos128,
            "sind": sin128,
            "identd": np.eye(128, dtype=np.float32),
            "onesd": np.ones((1, 128), dtype=np.float32),
        })
    return in_maps


def kernel(query, Wqkv, Wout):
    from concourse.bass_utils import run_bass_kernel_spmd

    nc = _build_program()
    in_maps = _host_prep(query, Wqkv, Wout)
    res = run_bass_kernel_spmd(nc, in_maps, core_ids=list(range(NCORES)))
    acc = np.zeros((EMBED, M), dtype=np.float32)
    for r in res.results:
        acc += r["out"]
    # out^T [E, b*T+t] -> [B, T, E] -> [T, B, E]
    full = acc.T.reshape(B, T, EMBED).transpose(1, 0, 2)
    return np.ascontiguousarray(full)



# revision 56
# speedup vs baseline: 1.0185x; 1.0185x over previous
"""Causal-self-attention (non-causal SDPA + RoPE) Bass kernel for 8 Trainium2 cores.

Sharding: head-parallel. 16 heads / 8 cores = 2 heads per core, all 4 batches.
Each core computes QKV projections for its 2 heads (tensor-parallel split of
Wqkv rows), RoPE, full attention for its 8 (batch, head) units, and a partial
output projection against its 128-column slice of Wout. The 8 partial outputs
are summed on the host (the all-reduce of the tensor-parallel out-proj).

Layouts on device (per core):
  xt      [1024, 8192]  X^T, f32r; column m = b*2048 + t (b-major)
  qt/kt   [128, 8192]   Q^T/K^T after RoPE; rows = 2 heads x 64 dims
  v       [128, 16*130] per batch: 16 s-tiles of [128s, 65+65] = [V_A|1 | V_B|1], bf16
  scores  S^T[s, t] via row-packed K=64 matmuls (2 heads concurrent on the PE)
  exp     ScalarE, scale=0.125 fused, no max-subtraction (scores ~ N(0,1))
  PV      attn^T[d, t] = [V|1].T @ E^T ; M=65 -> row 64 = softmax denominator
  outproj bf16: Wout slice^T as lhsT, scaled attn^T as rhs -> out^T [f, m]

Emission is software-pipelined per (batch, t-window): attention(b, tc) is
followed by one quarter of proj(b+1) and one quarter of outproj(b-1), so the
PE fills ScalarE-bound attention windows with projection work and ScalarE
never starves at batch boundaries.
"""

import numpy as np

EMBED = 1024
NUM_HEADS = 16
HEAD_DIM = 64
T = 2048
B = 4
NCORES = 8
M = T * B  # 8192
ROPE_BASE = 10000.0


def _build_program():
    import concourse.bass as bass  # noqa: F401
    import concourse.mybir as mybir
    import concourse.tile as tile
    from concourse import bacc

    dt = mybir.dt
    F32, F32R, BF16 = dt.float32, dt.float32r, dt.bfloat16
    AF = mybir.ActivationFunctionType

    nc = bacc.Bacc("TRN2", target_bir_lowering=False, debug=False,
                   num_devices=NCORES)

    xt = nc.dram_tensor("xt", [EMBED, M], F32R, kind="ExternalInput")
    wq = nc.dram_tensor("wq", [EMBED, 128], F32R, kind="ExternalInput")
    wk = nc.dram_tensor("wk", [EMBED, 128], F32R, kind="ExternalInput")
    wv = nc.dram_tensor("wv", [EMBED, 128], F32R, kind="ExternalInput")
    wo = nc.dram_tensor("wo", [128, EMBED], BF16, kind="ExternalInput")
    cosd = nc.dram_tensor("cosd", [128, T], F32, kind="ExternalInput")
    sind = nc.dram_tensor("sind", [128, T], F32, kind="ExternalInput")
    identd = nc.dram_tensor("identd", [128, 128], F32R, kind="ExternalInput")
    onesd = nc.dram_tensor("onesd", [1, 128], F32R, kind="ExternalInput")
    out = nc.dram_tensor("out", [EMBED, M], F32, kind="ExternalOutput")

    ST = 16            # s-tiles per batch (2048/128)
    VST = 130          # per-s-tile V columns: [V_A | 1 | V_B | 1]

    with tile.TileContext(nc) as tc:
        with (
            tc.tile_pool(name="const", bufs=1) as cpool,
            tc.tile_pool(name="xt", bufs=16) as xpool,
            tc.tile_pool(name="big", bufs=1) as big,
            tc.tile_pool(name="vt", bufs=2) as vtpool,
            tc.tile_pool(name="rt", bufs=2) as rtpool,
            tc.tile_pool(name="et", bufs=3) as epool,
            tc.tile_pool(name="sc", bufs=2) as scpool,
            tc.tile_pool(name="ob", bufs=3) as opool,
            tc.tile_pool(name="pp", bufs=2, space="PSUM") as pp,
            tc.tile_pool(name="ps", bufs=2, space="PSUM") as ps,
            tc.tile_pool(name="pa", bufs=2, space="PSUM") as pa,
        ):
            # ---- constants ----
            wq_sb = cpool.tile([128, 1024], F32R, tag="wq")
            wk_sb = cpool.tile([128, 1024], F32R, tag="wk")
            wv_sb = cpool.tile([128, 1024], F32R, tag="wv")
            for e in range(8):
                nc.sync.dma_start(wq_sb[:, e * 128:(e + 1) * 128],
                                  wq[e * 128:(e + 1) * 128, :])
                nc.sync.dma_start(wk_sb[:, e * 128:(e + 1) * 128],
                                  wk[e * 128:(e + 1) * 128, :])
                nc.sync.dma_start(wv_sb[:, e * 128:(e + 1) * 128],
                                  wv[e * 128:(e + 1) * 128, :])

            cos_sb = cpool.tile([128, T], F32, tag="cos")
            sin_sb = cpool.tile([128, T], F32, tag="sin")
            ident = cpool.tile([128, 128], F32R, tag="ident")
            ones1 = cpool.tile([1, 128], F32R, tag="ones")
            wo_sb = cpool.tile([128, 1024], BF16, tag="wo")

            def load_tables():
                nc.sync.dma_start(cos_sb[:], cosd[:])
                nc.sync.dma_start(sin_sb[:], sind[:])
                nc.sync.dma_start(ident[:], identd[:])
                nc.sync.dma_start(ones1[:], onesd[:])
                nc.sync.dma_start(wo_sb[:], wo[:])

            warm = cpool.tile([1, 128], F32, tag="warm")
            nc.scalar.activation(warm[:], ones1[:].bitcast(F32),
                                 AF.Exp, scale=0.0)
            qt_sb = big.tile([128, M], F32R, tag="qt")
            kt_sb = big.tile([128, M], F32R, tag="kt")
            v_sb = [big.tile([128, ST * VST], BF16, tag=f"v{b}", name=f"v_sb{b}")
                    for b in range(B)]
            attnS = {(b, g): rtpool.tile([128, 512], BF16, tag="attnS",
                                          name=f"attnS{b}_{g}")
                     for b in range(B) for g in range(4)}

            def load_x_half(h, mc):
                """Load 8 e-chunk tiles of X^T for 512 m-cols at h*1024+mc*512."""
                c0 = h * 1024 + mc * 512
                xts = [xpool.tile([128, 512], F32R, tag="xt",
                                  name=f"xt{h}_{mc}_{e}")
                       for e in range(8)]
                for e in range(8):
                    nc.sync.dma_start(xts[e][:],
                                      xt[e * 128:(e + 1) * 128, c0:c0 + 512])
                return xts

            def rope(p, dst, col0):
                """dst = cos*p + sin_eff*shift32(p), all [128, 512]."""
                pr = rtpool.tile([128, 512], F32, tag="proj_sb")
                nc.any.tensor_copy(pr[:], p[:])
                prs = rtpool.tile([128, 512], F32, tag="ropeshuf")
                for (ob, ib) in ((0, 32), (32, 0), (64, 96), (96, 64)):
                    nc.sync.dma_start(prs[ob:ob + 32, :], pr[ib:ib + 32, :])
                t2 = rtpool.tile([128, 512], F32, tag="ropetmp")
                tc0 = col0 % T
                nc.vector.tensor_mul(t2[:], prs[:], sin_sb[:, tc0:tc0 + 512])
                nc.vector.tensor_mul(dst, pr[:], cos_sb[:, tc0:tc0 + 512])
                nc.vector.tensor_add(dst, dst, t2[:])

            def proj_chunk(xts, h, mc):
                """Q/K/V projections + rope + V transpose for 512 m-columns."""
                b = h // 2
                col0 = h * 1024 + mc * 512
                for w_sb, dst in ((wq_sb, qt_sb), (wk_sb, kt_sb)):
                    p = pp.tile([128, 512], F32, tag="pp")
                    for e in range(8):
                        nc.tensor.matmul(
                            p[:], w_sb[:, e * 128:(e + 1) * 128], xts[e][:],
                            start=(e == 0), stop=(e == 7))
                    rope(p, dst[:, col0:col0 + 512], col0)
                p = pp.tile([128, 512], F32, tag="pp")
                for e in range(8):
                    nc.tensor.matmul(
                        p[:], wv_sb[:, e * 128:(e + 1) * 128], xts[e][:],
                        start=(e == 0), stop=(e == 7))
                vt = vtpool.tile([128, 512], F32R, tag="vt")
                nc.any.tensor_copy(vt[:], p[:])
                for k in range(4):
                    stt = (col0 % T) // 128 + k  # s-tile index 0..15
                    pt = pp.tile([128, 128], F32R, tag="pp")
                    nc.tensor.transpose(pt[:], vt[:, k * 128:(k + 1) * 128],
                                        ident[:])
                    # single strided copy: cols 0-63 -> +0, 64-127 -> +65
                    dstv = v_sb[b][:, stt * VST:stt * VST + 130]
                    nc.vector.tensor_copy(
                        dstv.rearrange("p (h c) -> p h c", c=65)[:, :, 0:64],
                        pt.rearrange("p (h c) -> p h c", c=64))

            def set_v_ones(b):
                nc.vector.memset(
                    v_sb[b].rearrange("p (s c) -> p s c", c=VST)[:, :, 64:65], 1.0)
                nc.vector.memset(
                    v_sb[b].rearrange("p (s c) -> p s c", c=VST)[:, :, 129:130], 1.0)

            def attention_tc(b, tcg):
                c0 = b * T + tcg * 512
                # swapped-operand PV: lhsT = e_t[s, t-chunk], rhs = [V|1]
                # -> attn[t, d] tiles of 65 cols (denominator at col 64,
                # per-partition).  One 2KB psum zero-region holds 4
                # interleaved chunk accumulations, so zero explicitly and
                # accumulate with start=False (commutative, reorder-safe).
                paA = pa.tile([128, 260], F32, tag="pa")
                paB = pa.tile([128, 260], F32, tag="pa")
                nc.vector.memset(paA[:], 0.0)
                nc.vector.memset(paB[:], 0.0)
                pax = (paA, paB)
                for st in range(ST):
                    s0 = b * T + st * 128
                    sab = ps.tile([128, 1024], F32, tag="sab")
                    nc.tensor.matmul(sab[:, 0:512],
                                     kt_sb[0:64, s0:s0 + 128],
                                     qt_sb[0:64, c0:c0 + 512],
                                     start=True, stop=True)
                    nc.tensor.matmul(sab[:, 512:1024],
                                     kt_sb[64:128, s0:s0 + 128],
                                     qt_sb[64:128, c0:c0 + 512],
                                     start=True, stop=True)
                    e_t = epool.tile([128, 1024], BF16, tag="et")
                    nc.scalar.activation(e_t[:], sab[:], AF.Exp, scale=0.125)
                    for h in range(2):
                        for j in range(4):
                            nc.tensor.matmul(
                                pax[h][:, j * 65:(j + 1) * 65],
                                e_t[:, h * 512 + j * 128:h * 512 + (j + 1) * 128],
                                v_sb[b][:, st * VST + h * 65:st * VST + (h + 1) * 65],
                                start=False, stop=False,
                                skip_group_check=True)
                # per-partition softmax scale, then PE transpose into the
                # [d2, t] attnS layout the out-proj consumes
                r4 = []
                for h in range(2):
                    r = scpool.tile([128, 4], F32, tag="rec", bufs=4,
                                    name=f"r4_{h}")
                    with nc.allow_low_precision(reason="softmax denom recip"):
                        nc.vector.reciprocal(
                            r[:], pax[h].rearrange(
                                "p (j c) -> p j c", c=65)[:, :, 64])
                    r4.append(r)
                for j in range(4):
                    sc = scpool.tile([128, 128], F32R, tag="bcs",
                                     name=f"sc{j}")
                    for h in range(2):
                        nc.vector.tensor_scalar_mul(
                            sc[:, h * 64:(h + 1) * 64],
                            pax[h][:, j * 65:j * 65 + 64],
                            r4[h][:, j:j + 1])
                    pt = pp.tile([128, 128], F32R, tag="pp", name=f"pt{j}")
                    nc.tensor.transpose(pt[:], sc[:], ident[:])
                    nc.vector.tensor_copy(
                        attnS[(b, tcg)][:, j * 128:(j + 1) * 128], pt[:])

            def outproj_q(b, tcg, evict_engine=None):
                for ft in range(8):
                    po = pp.tile([128, 512], F32, tag="pp")
                    nc.tensor.matmul(po[:],
                                     wo_sb[:, ft * 128:(ft + 1) * 128],
                                     attnS[(b, tcg)][:],
                                     start=True, stop=True)
                    o_sb = opool.tile([128, 512], F32, tag="ob")
                    if evict_engine is None:
                        nc.vector.tensor_copy(o_sb[:], po[:])
                    else:
                        evict_engine.activation(
                            o_sb[:], po[:],
                            mybir.ActivationFunctionType.Copy)
                    nc.sync.dma_start(
                        out[ft * 128:(ft + 1) * 128,
                            b * T + tcg * 512:b * T + (tcg + 1) * 512],
                        o_sb[:])

            # ---- software-pipelined emission ----
            for b in range(B):
                set_v_ones(b)
            first = load_x_half(0, 0)
            load_tables()
            proj_chunk(first, 0, 0)
            for g in range(1, 4):
                xts = load_x_half(g // 2, g % 2)
                proj_chunk(xts, g // 2, g % 2)
            prev = None
            for b in range(B):
                for tcg in range(4):
                    attention_tc(b, tcg)
                    if b + 1 < B:
                        h, mc = 2 * (b + 1) + tcg // 2, tcg % 2
                        xts = load_x_half(h, mc)
                        proj_chunk(xts, h, mc)
                    if prev is not None:
                        outproj_q(*prev)
                    prev = (b, tcg)
            outproj_q(*prev, evict_engine=nc.scalar)

    nc.compile()
    return nc


def _host_prep(query, Wqkv, Wout):
    import ml_dtypes

    q32 = np.asarray(query, dtype=np.float32)
    # [T, B, E] -> [E, B, T] -> [E, B*T]  (column = b*T + t)
    xt = np.ascontiguousarray(q32.transpose(2, 1, 0).reshape(EMBED, M))

    # rope tables, fp16-rounded like the reference
    theta = np.power(ROPE_BASE,
                     -np.arange(0, HEAD_DIM, 2, dtype=np.float32) / HEAD_DIM)
    m_th = np.arange(T, dtype=np.float32)[:, None] * theta[None, :]
    m_th = np.concatenate([m_th, m_th], axis=-1)          # [T, 64]
    cos = np.cos(m_th).astype(np.float16).astype(np.float32)
    sin = np.sin(m_th).astype(np.float16).astype(np.float32)
    cosT = cos.T                                          # [64, T]
    sin_eff = sin.T.copy()
    sin_eff[0:32] = -sin_eff[0:32]
    cos128 = np.ascontiguousarray(np.concatenate([cosT, cosT], axis=0))
    sin128 = np.ascontiguousarray(np.concatenate([sin_eff, sin_eff], axis=0))

    W = np.asarray(Wqkv, dtype=np.float32)
    Wo = np.asarray(Wout, dtype=np.float32)
    in_maps = []
    for c in range(NCORES):
        sl = slice(c * 128, (c + 1) * 128)
        in_maps.append({
            "xt": xt,
            "wq": np.ascontiguousarray(W[sl, :].T),
            "wk": np.ascontiguousarray(W[EMBED:][sl, :].T),
            "wv": np.ascontiguousarray(W[2 * EMBED:][sl, :].T),
            "wo": np.ascontiguousarray(Wo[:, sl].T).astype(ml_dtypes.bfloat16),
            "cosd": cos128,
            "sind": sin128,
            "identd": np.eye(128, dtype=np.float32),
            "onesd": np.ones((1, 128), dtype=np.float32),
        })
    return in_maps


def kernel(query, Wqkv, Wout):
    from concourse.bass_utils import run_bass_kernel_spmd

    nc = _build_program()
    in_maps = _host_prep(query, Wqkv, Wout)
    res = run_bass_kernel_spmd(nc, in_maps, core_ids=list(range(NCORES)))
    acc = np.zeros((EMBED, M), dtype=np.float32)
    for r in res.results:
        acc += r["out"]
    # out^T [E, b*T+t] -> [B, T, E] -> [T, B, E]
    full = acc.T.reshape(B, T, EMBED).transpose(1, 0, 2)
    return np.ascontiguousarray(full)



# revision 60
# speedup vs baseline: 1.0338x; 1.0151x over previous
"""Causal-self-attention (non-causal SDPA + RoPE) Bass kernel for 8 Trainium2 cores.

Sharding: head-parallel. 16 heads / 8 cores = 2 heads per core, all 4 batches.
Each core computes QKV projections for its 2 heads (tensor-parallel split of
Wqkv rows), RoPE, full attention for its 8 (batch, head) units, and a partial
output projection against its 128-column slice of Wout. The 8 partial outputs
are summed on the host (the all-reduce of the tensor-parallel out-proj).

Layouts on device (per core):
  xt      [1024, 8192]  X^T, f32r; column m = b*2048 + t (b-major)
  qt/kt   [128, 8192]   Q^T/K^T after RoPE; rows = 2 heads x 64 dims
  v       [128, 16*130] per batch: 16 s-tiles of [128s, 65+65] = [V_A|1 | V_B|1], bf16
  scores  S^T[s, t] via row-packed K=64 matmuls (2 heads concurrent on the PE)
  exp     ScalarE, scale=0.125 fused, no max-subtraction (scores ~ N(0,1))
  PV      attn^T[d, t] = [V|1].T @ E^T ; M=65 -> row 64 = softmax denominator
  outproj bf16: Wout slice^T as lhsT, scaled attn^T as rhs -> out^T [f, m]

Emission is software-pipelined per (batch, t-window): attention(b, tc) is
followed by one quarter of proj(b+1) and one quarter of outproj(b-1), so the
PE fills ScalarE-bound attention windows with projection work and ScalarE
never starves at batch boundaries.
"""

import numpy as np

EMBED = 1024
NUM_HEADS = 16
HEAD_DIM = 64
T = 2048
B = 4
NCORES = 8
M = T * B  # 8192
ROPE_BASE = 10000.0


def _build_program():
    import concourse.bass as bass  # noqa: F401
    import concourse.mybir as mybir
    import concourse.tile as tile
    from concourse import bacc

    dt = mybir.dt
    F32, F32R, BF16 = dt.float32, dt.float32r, dt.bfloat16
    AF = mybir.ActivationFunctionType

    nc = bacc.Bacc("TRN2", target_bir_lowering=False, debug=False,
                   num_devices=NCORES)

    xt = nc.dram_tensor("xt", [EMBED, M], F32R, kind="ExternalInput")
    wq = nc.dram_tensor("wq", [EMBED, 128], F32R, kind="ExternalInput")
    wk = nc.dram_tensor("wk", [EMBED, 128], F32R, kind="ExternalInput")
    wv = nc.dram_tensor("wv", [EMBED, 128], F32R, kind="ExternalInput")
    wo = nc.dram_tensor("wo", [128, EMBED], BF16, kind="ExternalInput")
    cosd = nc.dram_tensor("cosd", [128, T], F32, kind="ExternalInput")
    sind = nc.dram_tensor("sind", [128, T], F32, kind="ExternalInput")
    identd = nc.dram_tensor("identd", [128, 128], F32R, kind="ExternalInput")
    onesd = nc.dram_tensor("onesd", [1, 128], F32R, kind="ExternalInput")
    out = nc.dram_tensor("out", [EMBED, M], F32, kind="ExternalOutput")

    ST = 16            # s-tiles per batch (2048/128)
    VST = 130          # per-s-tile V columns: [V_A | 1 | V_B | 1]

    with tile.TileContext(nc) as tc:
        with (
            tc.tile_pool(name="const", bufs=1) as cpool,
            tc.tile_pool(name="xt", bufs=16) as xpool,
            tc.tile_pool(name="big", bufs=1) as big,
            tc.tile_pool(name="vt", bufs=2) as vtpool,
            tc.tile_pool(name="rt", bufs=2) as rtpool,
            tc.tile_pool(name="et", bufs=3) as epool,
            tc.tile_pool(name="sc", bufs=2) as scpool,
            tc.tile_pool(name="ob", bufs=3) as opool,
            tc.tile_pool(name="pp", bufs=2, space="PSUM") as pp,
            tc.tile_pool(name="ps", bufs=2, space="PSUM") as ps,
            tc.tile_pool(name="pa", bufs=2, space="PSUM") as pa,
        ):
            # ---- constants ----
            wq_sb = cpool.tile([128, 1024], F32R, tag="wq")
            wk_sb = cpool.tile([128, 1024], F32R, tag="wk")
            wv_sb = cpool.tile([128, 1024], F32R, tag="wv")
            for e in range(8):
                nc.sync.dma_start(wq_sb[:, e * 128:(e + 1) * 128],
                                  wq[e * 128:(e + 1) * 128, :])
                nc.sync.dma_start(wk_sb[:, e * 128:(e + 1) * 128],
                                  wk[e * 128:(e + 1) * 128, :])
                nc.sync.dma_start(wv_sb[:, e * 128:(e + 1) * 128],
                                  wv[e * 128:(e + 1) * 128, :])

            cos_sb = cpool.tile([128, T], F32, tag="cos")
            sin_sb = cpool.tile([128, T], F32, tag="sin")
            ident = cpool.tile([128, 128], F32R, tag="ident")
            ones1 = cpool.tile([1, 128], F32R, tag="ones")
            wo_sb = cpool.tile([128, 1024], BF16, tag="wo")

            def load_tables():
                nc.sync.dma_start(cos_sb[:], cosd[:])
                nc.sync.dma_start(sin_sb[:], sind[:])
                nc.sync.dma_start(ident[:], identd[:])
                nc.sync.dma_start(ones1[:], onesd[:])
                nc.sync.dma_start(wo_sb[:], wo[:])

            warm = cpool.tile([1, 128], F32, tag="warm")
            nc.scalar.activation(warm[:], ones1[:].bitcast(F32),
                                 AF.Exp, scale=0.0)
            qt_sb = big.tile([128, M], F32R, tag="qt")
            kt_sb = big.tile([128, M], F32R, tag="kt")
            v_sb = [big.tile([128, ST * VST], BF16, tag=f"v{b}", name=f"v_sb{b}")
                    for b in range(B)]
            attnS = {(b, g): rtpool.tile([128, 512], BF16, tag="attnS",
                                          name=f"attnS{b}_{g}")
                     for b in range(B) for g in range(4)}

            def load_x_half(h, mc):
                """Load 8 e-chunk tiles of X^T for 512 m-cols at h*1024+mc*512."""
                c0 = h * 1024 + mc * 512
                xts = [xpool.tile([128, 512], F32R, tag="xt",
                                  name=f"xt{h}_{mc}_{e}")
                       for e in range(8)]
                for e in range(8):
                    nc.sync.dma_start(xts[e][:],
                                      xt[e * 128:(e + 1) * 128, c0:c0 + 512])
                return xts

            def rope(p, dst, col0):
                """dst = cos*p + sin_eff*shift32(p), all [128, 512]."""
                pr = rtpool.tile([128, 512], F32, tag="proj_sb")
                # force DVE: on Activation this copy queues behind pending
                # exps inside the rope critical chain
                nc.vector.tensor_copy(pr[:], p[:])
                prs = rtpool.tile([128, 512], F32, tag="ropeshuf")
                for (ob, ib) in ((0, 32), (32, 0), (64, 96), (96, 64)):
                    nc.sync.dma_start(prs[ob:ob + 32, :], pr[ib:ib + 32, :])
                t2 = rtpool.tile([128, 512], F32, tag="ropetmp")
                tc0 = col0 % T
                nc.vector.tensor_mul(t2[:], prs[:], sin_sb[:, tc0:tc0 + 512])
                nc.vector.tensor_mul(dst, pr[:], cos_sb[:, tc0:tc0 + 512])
                nc.vector.tensor_add(dst, dst, t2[:])

            def proj_chunk(xts, h, mc):
                """Q/K/V projections + rope + V transpose for 512 m-columns."""
                b = h // 2
                col0 = h * 1024 + mc * 512
                for w_sb, dst in ((wq_sb, qt_sb), (wk_sb, kt_sb)):
                    p = pp.tile([128, 512], F32, tag="pp")
                    for e in range(8):
                        nc.tensor.matmul(
                            p[:], w_sb[:, e * 128:(e + 1) * 128], xts[e][:],
                            start=(e == 0), stop=(e == 7))
                    rope(p, dst[:, col0:col0 + 512], col0)
                p = pp.tile([128, 512], F32, tag="pp")
                for e in range(8):
                    nc.tensor.matmul(
                        p[:], wv_sb[:, e * 128:(e + 1) * 128], xts[e][:],
                        start=(e == 0), stop=(e == 7))
                vt = vtpool.tile([128, 512], F32R, tag="vt")
                nc.vector.tensor_copy(vt[:], p[:])
                for k in range(4):
                    stt = (col0 % T) // 128 + k  # s-tile index 0..15
                    pt = pp.tile([128, 128], F32R, tag="pp")
                    nc.tensor.transpose(pt[:], vt[:, k * 128:(k + 1) * 128],
                                        ident[:])
                    # single strided copy: cols 0-63 -> +0, 64-127 -> +65
                    dstv = v_sb[b][:, stt * VST:stt * VST + 130]
                    nc.vector.tensor_copy(
                        dstv.rearrange("p (h c) -> p h c", c=65)[:, :, 0:64],
                        pt.rearrange("p (h c) -> p h c", c=64))

            def set_v_ones(b):
                nc.vector.memset(
                    v_sb[b].rearrange("p (s c) -> p s c", c=VST)[:, :, 64:65], 1.0)
                nc.vector.memset(
                    v_sb[b].rearrange("p (s c) -> p s c", c=VST)[:, :, 129:130], 1.0)

            def attention_tc(b, tcg):
                c0 = b * T + tcg * 512
                # swapped-operand PV: lhsT = e_t[s, t-chunk], rhs = [V|1]
                # -> attn[t, d] tiles of 65 cols (denominator at col 64,
                # per-partition).  One 2KB psum zero-region holds 4
                # interleaved chunk accumulations, so zero explicitly and
                # accumulate with start=False (commutative, reorder-safe).
                paA = pa.tile([128, 260], F32, tag="pa")
                paB = pa.tile([128, 260], F32, tag="pa")
                nc.vector.memset(paA[:], 0.0)
                nc.vector.memset(paB[:], 0.0)
                pax = (paA, paB)
                for st in range(ST):
                    s0 = b * T + st * 128
                    sab = ps.tile([128, 1024], F32, tag="sab")
                    nc.tensor.matmul(sab[:, 0:512],
                                     kt_sb[0:64, s0:s0 + 128],
                                     qt_sb[0:64, c0:c0 + 512],
                                     start=True, stop=True)
                    nc.tensor.matmul(sab[:, 512:1024],
                                     kt_sb[64:128, s0:s0 + 128],
                                     qt_sb[64:128, c0:c0 + 512],
                                     start=True, stop=True)
                    e_t = epool.tile([128, 1024], BF16, tag="et")
                    nc.scalar.activation(e_t[:], sab[:], AF.Exp, scale=0.125)
                    for h in range(2):
                        for j in range(4):
                            nc.tensor.matmul(
                                pax[h][:, j * 65:(j + 1) * 65],
                                e_t[:, h * 512 + j * 128:h * 512 + (j + 1) * 128],
                                v_sb[b][:, st * VST + h * 65:st * VST + (h + 1) * 65],
                                start=False, stop=False,
                                skip_group_check=True)
                # per-partition softmax scale, then PE transpose into the
                # [d2, t] attnS layout the out-proj consumes
                r4 = []
                for h in range(2):
                    r = scpool.tile([128, 4], F32, tag="rec", bufs=4,
                                    name=f"r4_{h}")
                    with nc.allow_low_precision(reason="softmax denom recip"):
                        nc.vector.reciprocal(
                            r[:], pax[h].rearrange(
                                "p (j c) -> p j c", c=65)[:, :, 64])
                    r4.append(r)
                for j in range(4):
                    sc = scpool.tile([128, 128], F32R, tag="bcs",
                                     name=f"sc{j}")
                    for h in range(2):
                        nc.vector.tensor_scalar_mul(
                            sc[:, h * 64:(h + 1) * 64],
                            pax[h][:, j * 65:j * 65 + 64],
                            r4[h][:, j:j + 1])
                    pt = pp.tile([128, 128], F32R, tag="pp", name=f"pt{j}")
                    nc.tensor.transpose(pt[:], sc[:], ident[:])
                    nc.vector.tensor_copy(
                        attnS[(b, tcg)][:, j * 128:(j + 1) * 128], pt[:])

            def outproj_q(b, tcg, evict_engine=None):
                for ft in range(8):
                    po = pp.tile([128, 512], F32, tag="pp")
                    nc.tensor.matmul(po[:],
                                     wo_sb[:, ft * 128:(ft + 1) * 128],
                                     attnS[(b, tcg)][:],
                                     start=True, stop=True)
                    o_sb = opool.tile([128, 512], F32, tag="ob")
                    if evict_engine is None:
                        nc.vector.tensor_copy(o_sb[:], po[:])
                    else:
                        evict_engine.activation(
                            o_sb[:], po[:],
                            mybir.ActivationFunctionType.Copy)
                    nc.sync.dma_start(
                        out[ft * 128:(ft + 1) * 128,
                            b * T + tcg * 512:b * T + (tcg + 1) * 512],
                        o_sb[:])

            # ---- software-pipelined emission ----
            for b in range(B):
                set_v_ones(b)
            first = load_x_half(0, 0)
            load_tables()
            proj_chunk(first, 0, 0)
            for g in range(1, 4):
                xts = load_x_half(g // 2, g % 2)
                proj_chunk(xts, g // 2, g % 2)
            prev = None
            for b in range(B):
                for tcg in range(4):
                    attention_tc(b, tcg)
                    if b + 1 < B:
                        h, mc = 2 * (b + 1) + tcg // 2, tcg % 2
                        xts = load_x_half(h, mc)
                        proj_chunk(xts, h, mc)
                    if prev is not None:
                        outproj_q(*prev)
                    prev = (b, tcg)
            outproj_q(*prev, evict_engine=nc.scalar)

    nc.compile()
    return nc


def _host_prep(query, Wqkv, Wout):
    import ml_dtypes

    q32 = np.asarray(query, dtype=np.float32)
    # [T, B, E] -> [E, B, T] -> [E, B*T]  (column = b*T + t)
    xt = np.ascontiguousarray(q32.transpose(2, 1, 0).reshape(EMBED, M))

    # rope tables, fp16-rounded like the reference
    theta = np.power(ROPE_BASE,
                     -np.arange(0, HEAD_DIM, 2, dtype=np.float32) / HEAD_DIM)
    m_th = np.arange(T, dtype=np.float32)[:, None] * theta[None, :]
    m_th = np.concatenate([m_th, m_th], axis=-1)          # [T, 64]
    cos = np.cos(m_th).astype(np.float16).astype(np.float32)
    sin = np.sin(m_th).astype(np.float16).astype(np.float32)
    cosT = cos.T                                          # [64, T]
    sin_eff = sin.T.copy()
    sin_eff[0:32] = -sin_eff[0:32]
    cos128 = np.ascontiguousarray(np.concatenate([cosT, cosT], axis=0))
    sin128 = np.ascontiguousarray(np.concatenate([sin_eff, sin_eff], axis=0))

    W = np.asarray(Wqkv, dtype=np.float32)
    Wo = np.asarray(Wout, dtype=np.float32)
    in_maps = []
    for c in range(NCORES):
        sl = slice(c * 128, (c + 1) * 128)
        in_maps.append({
            "xt": xt,
            "wq": np.ascontiguousarray(W[sl, :].T),
            "wk": np.ascontiguousarray(W[EMBED:][sl, :].T),
            "wv": np.ascontiguousarray(W[2 * EMBED:][sl, :].T),
            "wo": np.ascontiguousarray(Wo[:, sl].T).astype(ml_dtypes.bfloat16),
            "cosd": cos128,
            "sind": sin128,
            "identd": np.eye(128, dtype=np.float32),
            "onesd": np.ones((1, 128), dtype=np.float32),
        })
    return in_maps


def kernel(query, Wqkv, Wout):
    from concourse.bass_utils import run_bass_kernel_spmd

    nc = _build_program()
    in_maps = _host_prep(query, Wqkv, Wout)
    res = run_bass_kernel_spmd(nc, in_maps, core_ids=list(range(NCORES)))
    acc = np.zeros((EMBED, M), dtype=np.float32)
    for r in res.results:
        acc += r["out"]
    # out^T [E, b*T+t] -> [B, T, E] -> [T, B, E]
    full = acc.T.reshape(B, T, EMBED).transpose(1, 0, 2)
    return np.ascontiguousarray(full)

